# revision 42
# baseline (speedup 1.0000x reference)
"""Trainium2 Bass kernel for CascadedLoRALinear:
    out = x @ W^T + b + 4.0 * (x @ A^T) @ B^T
        + 2.0 * ((((x @ A1^T) @ A2^T) @ B1^T) @ B2^T)

Sharding: data-parallel over tokens (batch*seq = 8192 -> 1024/core on 8 cores),
all weights replicated.

The base matmul runs in fp8 (e4m3) DoubleRow perf mode: two K=128 chunks per
matmul instruction at 0.5 cycles/row -> 2x the fp16 PE throughput. Precision
holds because the LoRA1 term (scale 4.0, kept in fp16) dominates the output
(std ~5.1 vs base ~1.3); the fp8 base contributes ~1% relative error overall.
W sits in e4m3's subnormal range (std 0.02), so the host scales W by 64 before
quantizing; Rc/bias are pre-scaled by 64 to match, and the PSUM->SBUF copy
applies the 1/64 (activation Copy with scale).

Host-side prep (outside the HW kernel): weights pre-transposed, W^T scaled and
cast to fp8e4, the rest fp16; the three rank-space contributions fused into
one [128, OUT] matrix Rc so the whole LoRA correction + bias is ONE extra
K=128 matmul per output tile:
    rows  0:64  = 64 * 4.0 * B^T   (applied to h1 = A @ x^T)
    rows 64:96  = 64 * 2.0 * B2^T  (applied to h4 = B1 @ A2 @ A1 @ x^T)
    row     96  = 64 * b           (applied to a constant ones row)
    rows 97:128 = 0

The cascade B1@A2@A1 is folded on the host into one [32, IN] weight (a
weights-only algebraic fold), so the device chain [A; B1A2A1] @ x^T directly
produces both h1 and h4 — no on-device cascade at all.

Device per core:
    phase A: h14 = [A; B1A2A1] @ x^T (fp16, one 96-wide matmul chain over
             K=4096), DVE casts each landed x chunk to fp8 pair-tiles, early
             fp8 DoubleRow base matmuls on n=0 keep the PE dense while W
             streams; h14 is copied into hcomb [128, TOK].
    phase B: per (128-token, 512-out) tile: 16 accumulating DoubleRow K=256
             fp8 matmuls of x^T @ W^T + 1 fused fp16 matmul hcomb^T @ Rc,
             PSUM f32 at 64x scale, scaled copy to SBUF, DMA to out.
"""

import sys

import numpy as np

try:
    import concourse.bass  # noqa: F401
except ImportError:
    sys.path.insert(0, "/opt/trn_rl_repo")

import ml_dtypes
import concourse.mybir as mybir
import concourse.tile as tile
from concourse import bacc
from concourse.bass_utils import run_bass_kernel_spmd

F16 = np.float16
F8 = ml_dtypes.float8_e4m3

N_CORES = 8
NTOK, IN, OUT = 8192, 4096, 4096
TOK = NTOK // N_CORES          # 1024 tokens per core
P = 128
KO = IN // P                   # 32 contraction chunks
KO2 = KO // 2                  # 16 fp8 DoubleRow pair-chunks
NTILE = 512
NT = OUT // NTILE              # 8 out tiles
TT = TOK // P                  # 8 token tiles
MT = TOK // NTILE              # 2 token macro-tiles (chain phase)
S1, S2 = 4.0, 2.0
WSC = 64.0                     # fp8 pre-scale on W (and Rc); undone on copy-out

_nc_cache = None


def _build():
    nc = bacc.Bacc(None, target_bir_lowering=False)
    f16 = mybir.dt.float16
    f8 = mybir.dt.float8e4
    f32 = mybir.dt.float32
    DR = mybir.MatmulPerfMode.DoubleRow

    xT_d = nc.declare_dram_parameter("xT", [P, KO, TOK], f16, isOutput=False)
    W_d = nc.declare_dram_parameter("Wd", [NT, P, KO, NTILE], f8, isOutput=False)
    # chain weights zero-padded 96 -> 128 stationary columns: full-tile
    # matmuls avoid the partial-tile penalty (~8ns/mm measured at 96)
    ATc_d = nc.declare_dram_parameter("ATc", [P, KO, P], f16, isOutput=False)
    Rc_d = nc.declare_dram_parameter("Rc", [P, OUT], f16, isOutput=False)
    out_d = nc.declare_dram_parameter("out", [TOK, OUT], f16, isOutput=True)

    KG = 4                 # W ko-chunks per sub-tile DMA (2 DoubleRow pairs)
    NSUB = KO // KG        # 8 sub-tiles per n-tile

    with tile.TileContext(nc) as tc:
        with (
            tc.tile_pool(name="persist", bufs=1) as persist,
            tc.tile_pool(name="wpool", bufs=2 * NSUB) as wpool,
            tc.tile_pool(name="outpool", bufs=3) as outpool,
            tc.tile_pool(name="psum_out", bufs=6, space="PSUM") as psum_out,
            tc.tile_pool(name="psum_h", bufs=1, space="PSUM") as psum_h,
        ):
            # PE warmup: junk matmuls (never read) sized so the HAM clock
            # gate (1.2 -> 2.4 GHz after ~3.4us of PE-busy time) opens right
            # as the first x chunk lands (~10.5us; HBM fair-share across the
            # ~45 upfront DMAs delays first arrival). The junk sits entirely
            # inside the PE's data-wait window, so the ramp completes for
            # free and the real stream runs at full clock from its first op.
            warm_sb = persist.tile([P, P], f16)
            nc.vector.memset(warm_sb[:], 0.0)
            warm_ps = psum_out.tile([P, P], f32, tag="po", name="warm_ps")
            for _ in range(24):
                nc.tensor.matmul(warm_ps[:], warm_sb[:], warm_sb[:],
                                 start=True, stop=True)

            def w_subtiles(n):
                subs = []
                for s in range(NSUB):
                    wt = wpool.tile([P, KG, NTILE], f8, tag="w")
                    nc.sync.dma_start(
                        out=wt[:], in_=W_d[n, :, s * KG:(s + 1) * KG, :])
                    subs.append(wt)
                return subs

            # x^T loaded as 32 independent per-ko chunk tiles and [A;A1]^T
            # as 8 sub-tiles, DMA-issued interleaved with n=0's W sub-tiles
            # so the during-load compute is fed in lockstep
            xk = [persist.tile([P, TOK], f16, tag=f"x{ko}", name=f"xk{ko}")
                  for ko in range(KO)]
            atq = [persist.tile([P, KG, P], f16, tag=f"at{s}", name=f"atq{s}")
                   for s in range(NSUB)]
            nc.sync.dma_start(out=atq[0][:], in_=ATc_d[:, 0:KG, :])
            for ko in range(4):
                nc.sync.dma_start(out=xk[ko][:], in_=xT_d[:, ko, :])
            w0 = []
            for s in range(NSUB):
                wt = wpool.tile([P, KG, NTILE], f8, tag="w", name=f"w0_{s}")
                nc.sync.dma_start(
                    out=wt[:], in_=W_d[0, :, s * KG:(s + 1) * KG, :])
                w0.append(wt)
                if s > 0:
                    nc.sync.dma_start(
                        out=atq[s][:], in_=ATc_d[:, s * KG:(s + 1) * KG, :])
                for ko in range(max(s * KG, 4), (s + 1) * KG):
                    nc.sync.dma_start(out=xk[ko][:], in_=xT_d[:, ko, :])

            # fp8 x pair-tiles, DVE-cast from the f16 chunks as they land
            x8k = [persist.tile([P, 2, TOK], f8, tag=f"x8_{kp}",
                                name=f"x8k{kp}")
                   for kp in range(KO2)]

            rcs = persist.tile([P, OUT], f16)
            nc.sync.dma_start(out=rcs[:], in_=Rc_d[:])

            hcomb = persist.tile([P, TOK], f16)
            nc.any.memset(hcomb[96:128, :], 0.0)
            nc.any.memset(hcomb[96:97, :], 1.0)

            def finalize(po, tt, n, ocols):
                trows = slice(tt * P, (tt + 1) * P)
                nc.tensor.matmul(po[:], hcomb[:, trows], rcs[:, ocols],
                                 start=False, stop=True)
                # f16 output (host upcasts): halves out-DMA traffic; rounding
                # adds ~3e-4 relative error vs the 9e-3 fp8 base term
                ot = outpool.tile([P, NTILE], f16)
                nc.scalar.activation(ot[:], po[:],
                                     mybir.ActivationFunctionType.Copy,
                                     scale=1.0 / WSC)
                nc.sync.dma_start(out=out_d[trows, ocols], in_=ot[:])

            def wpair(wsub, kp):
                ko = 2 * kp
                return wsub[ko // KG][:, ko % KG:ko % KG + 2, :]

            # during-load phase: rank chain + fp8 casts interleaved with the
            # first 5 token-groups of n=0, per pair-chunk, to keep the PE
            # dense while x^T / W stream in
            NEARLY = 5
            h12 = [psum_h.tile([P, NTILE], f32, tag=f"h12_{mt}", name=f"h12_{mt}")
                   for mt in range(MT)]
            po_early = [psum_out.tile([P, NTILE], f32, tag="po", name=f"poe{tt}")
                        for tt in range(NEARLY)]
            oc0 = slice(0, NTILE)
            LAGP = 2  # early-B matmuls trail the chain by LAGP pair-chunks so
                      # the in-order PE stream never stalls on W / the casts

            def early_b(kp):
                for tt in range(NEARLY):
                    trows = slice(tt * P, (tt + 1) * P)
                    nc.tensor.matmul(
                        po_early[tt][:], x8k[kp][:, :, trows], wpair(w0, kp),
                        start=(kp == 0), stop=False, perf_mode=DR,
                    )

            for kp in range(KO2):
                for j in range(2):
                    ko = 2 * kp + j
                    for mt in range(MT):
                        cols = slice(mt * NTILE, (mt + 1) * NTILE)
                        nc.tensor.matmul(
                            h12[mt][:], atq[ko // KG][:, ko % KG, :],
                            xk[ko][:, cols],
                            start=(ko == 0), stop=(ko == KO - 1),
                        )
                    nc.vector.tensor_copy(out=x8k[kp][:, j, :], in_=xk[ko][:])
                if kp >= LAGP:
                    early_b(kp - LAGP)
                else:
                    # dependency-free filler keeps the PE hot while the next
                    # x chunk streams in (early-B hasn't started yet)
                    for _ in range(4):
                        nc.tensor.matmul(warm_ps[:], warm_sb[:], warm_sb[:],
                                         start=True, stop=True)
            for kp in range(KO2 - LAGP, KO2):
                early_b(kp)

            # chain tail: hcomb assembly per macro-tile (the cascade was
            # folded into the chain weights host-side)
            for mt in range(MT):
                cols = slice(mt * NTILE, (mt + 1) * NTILE)
                nc.vector.tensor_copy(out=hcomb[0:96, cols],
                                      in_=h12[mt][0:96, :])

            def base_mms(po, tt, wsub):
                trows = slice(tt * P, (tt + 1) * P)
                for kp in range(KO2):
                    nc.tensor.matmul(
                        po[:], x8k[kp][:, :, trows], wpair(wsub, kp),
                        start=(kp == 0), stop=False, perf_mode=DR,
                    )

            # tt=NEARLY of n=0 first: its inputs are resident, so it covers
            # the PE stall while the chain tail's DVE copies assemble hcomb
            po4 = psum_out.tile([P, NTILE], f32, tag="po")
            base_mms(po4, NEARLY, w0)
            for tt in range(NEARLY):
                finalize(po_early[tt], tt, 0, oc0)
            finalize(po4, NEARLY, 0, oc0)

            # the very last tile drains through two half-width PSUM groups:
            # same PE cycles, but the first half's copy+DMA overlap the
            # second half's matmuls, shortening the serial tail
            def last_tile(tt, n, ocols):
                HN = NTILE // 2
                trows = slice(tt * P, (tt + 1) * P)
                for h in range(2):
                    po = psum_out.tile([P, HN], f32, tag="po")
                    hs = slice(h * HN, (h + 1) * HN)
                    for kp in range(KO2):
                        ko = 2 * kp
                        nc.tensor.matmul(
                            po[:], x8k[kp][:, :, trows],
                            wsub[ko // KG][:, ko % KG:ko % KG + 2, hs],
                            start=(kp == 0), stop=False, perf_mode=DR,
                        )
                    ocs = slice(ocols.start + h * HN, ocols.start + (h + 1) * HN)
                    nc.tensor.matmul(po[:], hcomb[:, trows], rcs[:, ocs],
                                     start=False, stop=True)
                    ot = outpool.tile([P, HN], f16)
                    nc.scalar.activation(ot[:], po[:],
                                         mybir.ActivationFunctionType.Copy,
                                         scale=1.0 / WSC)
                    nc.sync.dma_start(out=out_d[trows, ocs], in_=ot[:])

            # phase B: remaining groups. W for n+1 is prefetched right after
            # the first tile of n, so each n-transition never waits on DMA.
            wsub = w0
            for n in range(NT):
                ocols = slice(n * NTILE, (n + 1) * NTILE)
                first = True
                for tt in range(NEARLY + 1 if n == 0 else 0, TT):
                    if n == NT - 1 and tt == TT - 1:
                        last_tile(tt, n, ocols)
                        continue
                    po = psum_out.tile([P, NTILE], f32, tag="po")
                    base_mms(po, tt, wsub)
                    finalize(po, tt, n, ocols)
                    if first and n + 1 < NT:
                        wsub_next = w_subtiles(n + 1)
                        first = False
                wsub = wsub_next

    nc.compile()
    return nc


def _get_nc():
    global _nc_cache
    if _nc_cache is None:
        _nc_cache = _build()
    return _nc_cache


def make_in_maps(x, W, b, A, B, A1, A2, B1, B2):
    """Host-side shard + pack. Returns per-core in_maps for run_bass_kernel_spmd."""
    x = np.ascontiguousarray(np.asarray(x, np.float32)).reshape(NTOK, IN)
    W = np.asarray(W, np.float32)
    b = np.asarray(b, np.float32)
    A = np.asarray(A, np.float32)
    B = np.asarray(B, np.float32)
    A1 = np.asarray(A1, np.float32)
    A2 = np.asarray(A2, np.float32)
    B1 = np.asarray(B1, np.float32)
    B2 = np.asarray(B2, np.float32)

    # W^T [IN, OUT] -> [NT, P, KO, NTILE] so each n-tile DMA is contiguous;
    # scaled by 64 out of e4m3's subnormal range, quantized to fp8
    Wd = np.ascontiguousarray(
        (W.T * WSC).astype(F8).reshape(KO, P, NT, NTILE).transpose(2, 1, 0, 3)
    )
    # cascade fold: B1@A2@A1 [32, IN] in f64, so the chain emits h4 directly;
    # zero-padded to 128 rank rows for full-tile chain matmuls
    AB1 = (B1.astype(np.float64) @ A2.astype(np.float64)
           @ A1.astype(np.float64)).astype(np.float32)
    ATc = np.ascontiguousarray(
        np.concatenate([A.T, AB1.T, np.zeros((IN, 32), np.float32)], axis=1)
        .astype(F16).reshape(KO, P, P).transpose(1, 0, 2)
    )
    Rc = np.zeros((P, OUT), F16)
    Rc[0:64] = (WSC * S1 * B.T).astype(F16)
    Rc[64:96] = (WSC * S2 * B2.T).astype(F16)
    Rc[96] = (WSC * b).astype(F16)

    in_maps = []
    for c in range(N_CORES):
        xs = x[c * TOK:(c + 1) * TOK]                      # [TOK, IN]
        xT = np.ascontiguousarray(
            xs.T.astype(F16).reshape(KO, P, TOK).transpose(1, 0, 2)
        )
        in_maps.append({"xT": xT, "Wd": Wd, "ATc": ATc, "Rc": Rc})
    return in_maps


def kernel(x, W, b, A, B, A1, A2, B1, B2):
    nc = _get_nc()
    in_maps = make_in_maps(x, W, b, A, B, A1, A2, B1, B2)
    res = run_bass_kernel_spmd(nc, in_maps, core_ids=list(range(N_CORES)))
    out = np.concatenate(
        [res.results[c]["out"].astype(np.float32) for c in range(N_CORES)],
        axis=0)
    return out.reshape(4, 2048, OUT)


# revision 45
# speedup vs baseline: 1.0083x; 1.0083x over previous
"""Trainium2 Bass kernel for CascadedLoRALinear:
    out = x @ W^T + b + 4.0 * (x @ A^T) @ B^T
        + 2.0 * ((((x @ A1^T) @ A2^T) @ B1^T) @ B2^T)

Sharding: data-parallel over tokens (batch*seq = 8192 -> 1024/core on 8 cores),
all weights replicated.

The base matmul runs in fp8 (e4m3) DoubleRow perf mode: two K=128 chunks per
matmul instruction at 0.5 cycles/row -> 2x the fp16 PE throughput. Precision
holds because the LoRA1 term (scale 4.0, kept in fp16) dominates the output
(std ~5.1 vs base ~1.3); the fp8 base contributes ~1% relative error overall.
W sits in e4m3's subnormal range (std 0.02), so the host scales W by 64 before
quantizing; Rc/bias are pre-scaled by 64 to match, and the PSUM->SBUF copy
applies the 1/64 (activation Copy with scale).

Host-side prep (outside the HW kernel): weights pre-transposed, W^T scaled and
cast to fp8e4, the rest fp16; the three rank-space contributions fused into
one [128, OUT] matrix Rc so the whole LoRA correction + bias is ONE extra
K=128 matmul per output tile:
    rows  0:64  = 64 * 4.0 * B^T   (applied to h1 = A @ x^T)
    rows 64:96  = 64 * 2.0 * B2^T  (applied to h4 = B1 @ A2 @ A1 @ x^T)
    row     96  = 64 * b           (applied to a constant ones row)
    rows 97:128 = 0

The cascade B1@A2@A1 is folded on the host into one [32, IN] weight (a
weights-only algebraic fold), so the device chain [A; B1A2A1] @ x^T directly
produces both h1 and h4 — no on-device cascade at all.

Device per core:
    phase A: h14 = [A; B1A2A1] @ x^T (fp16, one 96-wide matmul chain over
             K=4096), DVE casts each landed x chunk to fp8 pair-tiles, early
             fp8 DoubleRow base matmuls on n=0 keep the PE dense while W
             streams; h14 is copied into hcomb [128, TOK].
    phase B: per (128-token, 512-out) tile: 16 accumulating DoubleRow K=256
             fp8 matmuls of x^T @ W^T + 1 fused fp16 matmul hcomb^T @ Rc,
             PSUM f32 at 64x scale, scaled copy to SBUF, DMA to out.
"""

import sys

import numpy as np

try:
    import concourse.bass  # noqa: F401
except ImportError:
    sys.path.insert(0, "/opt/trn_rl_repo")

import ml_dtypes
import concourse.mybir as mybir
import concourse.tile as tile
from concourse import bacc
from concourse.bass_utils import run_bass_kernel_spmd

F16 = np.float16
F8 = ml_dtypes.float8_e4m3

N_CORES = 8
NTOK, IN, OUT = 8192, 4096, 4096
TOK = NTOK // N_CORES          # 1024 tokens per core
P = 128
KO = IN // P                   # 32 contraction chunks
KO2 = KO // 2                  # 16 fp8 DoubleRow pair-chunks
NTILE = 512
NT = OUT // NTILE              # 8 out tiles
TT = TOK // P                  # 8 token tiles
MT = TOK // NTILE              # 2 token macro-tiles (chain phase)
S1, S2 = 4.0, 2.0
WSC = 64.0                     # fp8 pre-scale on W (and Rc); undone on copy-out

_nc_cache = None


def _build():
    nc = bacc.Bacc(None, target_bir_lowering=False)
    f16 = mybir.dt.float16
    f8 = mybir.dt.float8e4
    f32 = mybir.dt.float32
    DR = mybir.MatmulPerfMode.DoubleRow

    xT_d = nc.declare_dram_parameter("xT", [P, KO, TOK], f16, isOutput=False)
    W_d = nc.declare_dram_parameter("Wd", [NT, P, KO, NTILE], f8, isOutput=False)
    # chain weights zero-padded 96 -> 128 stationary columns: full-tile
    # matmuls avoid the partial-tile penalty (~8ns/mm measured at 96)
    ATc_d = nc.declare_dram_parameter("ATc", [P, KO, P], f16, isOutput=False)
    Rc_d = nc.declare_dram_parameter("Rc", [P, OUT], f16, isOutput=False)
    out_d = nc.declare_dram_parameter("out", [TOK, OUT], f16, isOutput=True)

    KG = 4                 # W ko-chunks per sub-tile DMA (2 DoubleRow pairs)
    NSUB = KO // KG        # 8 sub-tiles per n-tile

    with tile.TileContext(nc) as tc:
        with (
            tc.tile_pool(name="persist", bufs=1) as persist,
            tc.tile_pool(name="wpool", bufs=2 * NSUB) as wpool,
            tc.tile_pool(name="outpool", bufs=3) as outpool,
            tc.tile_pool(name="psum_out", bufs=6, space="PSUM") as psum_out,
            tc.tile_pool(name="psum_h", bufs=1, space="PSUM") as psum_h,
        ):
            # PE warmup: junk matmuls (never read) sized so the HAM clock
            # gate (1.2 -> 2.4 GHz after ~3.4us of PE-busy time) opens right
            # as the first x chunk lands (~10.5us; HBM fair-share across the
            # ~45 upfront DMAs delays first arrival). The junk sits entirely
            # inside the PE's data-wait window, so the ramp completes for
            # free and the real stream runs at full clock from its first op.
            warm_sb = persist.tile([P, P], f16)
            nc.vector.memset(warm_sb[:], 0.0)
            warm_ps = psum_out.tile([P, P], f32, tag="po", name="warm_ps")
            for _ in range(24):
                nc.tensor.matmul(warm_ps[:], warm_sb[:], warm_sb[:],
                                 start=True, stop=True)

            def w_subtiles(n):
                subs = []
                for s in range(NSUB):
                    wt = wpool.tile([P, KG, NTILE], f8, tag="w")
                    nc.sync.dma_start(
                        out=wt[:], in_=W_d[n, :, s * KG:(s + 1) * KG, :])
                    subs.append(wt)
                return subs

            # x^T loaded as 32 independent per-ko chunk tiles and [A;A1]^T
            # as 8 sub-tiles, DMA-issued interleaved with n=0's W sub-tiles
            # so the during-load compute is fed in lockstep
            xk = [persist.tile([P, TOK], f16, tag=f"x{ko}", name=f"xk{ko}")
                  for ko in range(KO)]
            atq = [persist.tile([P, KG, P], f16, tag=f"at{s}", name=f"atq{s}")
                   for s in range(NSUB)]
            nc.sync.dma_start(out=atq[0][:], in_=ATc_d[:, 0:KG, :])
            for ko in range(4):
                nc.sync.dma_start(out=xk[ko][:], in_=xT_d[:, ko, :])
            w0 = []
            for s in range(NSUB):
                wt = wpool.tile([P, KG, NTILE], f8, tag="w", name=f"w0_{s}")
                nc.sync.dma_start(
                    out=wt[:], in_=W_d[0, :, s * KG:(s + 1) * KG, :])
                w0.append(wt)
                if s > 0:
                    nc.sync.dma_start(
                        out=atq[s][:], in_=ATc_d[:, s * KG:(s + 1) * KG, :])
                for ko in range(max(s * KG, 4), (s + 1) * KG):
                    nc.sync.dma_start(out=xk[ko][:], in_=xT_d[:, ko, :])

            # fp8 x pair-tiles, DVE-cast from the f16 chunks as they land
            x8k = [persist.tile([P, 2, TOK], f8, tag=f"x8_{kp}",
                                name=f"x8k{kp}")
                   for kp in range(KO2)]

            # Rc isn't needed until the first finalize (~45us): its DMA is
            # issued mid-chain-loop, off the bandwidth-tight early window
            rcs = persist.tile([P, OUT], f16)

            hcomb = persist.tile([P, TOK], f16)
            nc.any.memset(hcomb[96:128, :], 0.0)
            nc.any.memset(hcomb[96:97, :], 1.0)

            def finalize(po, tt, n, ocols):
                trows = slice(tt * P, (tt + 1) * P)
                nc.tensor.matmul(po[:], hcomb[:, trows], rcs[:, ocols],
                                 start=False, stop=True)
                # f16 output (host upcasts): halves out-DMA traffic; rounding
                # adds ~3e-4 relative error vs the 9e-3 fp8 base term
                ot = outpool.tile([P, NTILE], f16)
                nc.scalar.activation(ot[:], po[:],
                                     mybir.ActivationFunctionType.Copy,
                                     scale=1.0 / WSC)
                nc.sync.dma_start(out=out_d[trows, ocols], in_=ot[:])

            def wpair(wsub, kp):
                ko = 2 * kp
                return wsub[ko // KG][:, ko % KG:ko % KG + 2, :]

            # during-load phase: rank chain + fp8 casts interleaved with the
            # first 5 token-groups of n=0, per pair-chunk, to keep the PE
            # dense while x^T / W stream in
            NEARLY = 5
            h12 = [psum_h.tile([P, NTILE], f32, tag=f"h12_{mt}", name=f"h12_{mt}")
                   for mt in range(MT)]
            po_early = [psum_out.tile([P, NTILE], f32, tag="po", name=f"poe{tt}")
                        for tt in range(NEARLY)]
            oc0 = slice(0, NTILE)
            LAGP = 2  # early-B matmuls trail the chain by LAGP pair-chunks so
                      # the in-order PE stream never stalls on W / the casts

            def early_b(kp):
                for tt in range(NEARLY):
                    trows = slice(tt * P, (tt + 1) * P)
                    nc.tensor.matmul(
                        po_early[tt][:], x8k[kp][:, :, trows], wpair(w0, kp),
                        start=(kp == 0), stop=False, perf_mode=DR,
                    )

            for kp in range(KO2):
                if kp == 10:
                    nc.sync.dma_start(out=rcs[:], in_=Rc_d[:])
                for j in range(2):
                    ko = 2 * kp + j
                    for mt in range(MT):
                        cols = slice(mt * NTILE, (mt + 1) * NTILE)
                        nc.tensor.matmul(
                            h12[mt][:], atq[ko // KG][:, ko % KG, :],
                            xk[ko][:, cols],
                            start=(ko == 0), stop=(ko == KO - 1),
                        )
                    nc.vector.tensor_copy(out=x8k[kp][:, j, :], in_=xk[ko][:])
                if kp >= LAGP:
                    early_b(kp - LAGP)
                else:
                    # dependency-free filler keeps the PE hot while the next
                    # x chunk streams in (early-B hasn't started yet)
                    for _ in range(4):
                        nc.tensor.matmul(warm_ps[:], warm_sb[:], warm_sb[:],
                                         start=True, stop=True)
            for kp in range(KO2 - LAGP, KO2):
                early_b(kp)

            # chain tail: hcomb assembly per macro-tile (the cascade was
            # folded into the chain weights host-side)
            for mt in range(MT):
                cols = slice(mt * NTILE, (mt + 1) * NTILE)
                nc.vector.tensor_copy(out=hcomb[0:96, cols],
                                      in_=h12[mt][0:96, :])

            def base_mms(po, tt, wsub):
                trows = slice(tt * P, (tt + 1) * P)
                for kp in range(KO2):
                    nc.tensor.matmul(
                        po[:], x8k[kp][:, :, trows], wpair(wsub, kp),
                        start=(kp == 0), stop=False, perf_mode=DR,
                    )

            # tt=NEARLY of n=0 first: its inputs are resident, so it covers
            # the PE stall while the chain tail's DVE copies assemble hcomb
            po4 = psum_out.tile([P, NTILE], f32, tag="po")
            base_mms(po4, NEARLY, w0)
            for tt in range(NEARLY):
                finalize(po_early[tt], tt, 0, oc0)
            finalize(po4, NEARLY, 0, oc0)

            # the very last tile drains through two half-width PSUM groups:
            # same PE cycles, but the first half's copy+DMA overlap the
            # second half's matmuls, shortening the serial tail
            def last_tile(tt, n, ocols):
                HN = NTILE // 2
                trows = slice(tt * P, (tt + 1) * P)
                ot = outpool.tile([P, NTILE], f16)
                for h in range(2):
                    po = psum_out.tile([P, HN], f32, tag="po")
                    hs = slice(h * HN, (h + 1) * HN)
                    for kp in range(KO2):
                        ko = 2 * kp
                        nc.tensor.matmul(
                            po[:], x8k[kp][:, :, trows],
                            wsub[ko // KG][:, ko % KG:ko % KG + 2, hs],
                            start=(kp == 0), stop=False, perf_mode=DR,
                        )
                    ocs = slice(ocols.start + h * HN, ocols.start + (h + 1) * HN)
                    nc.tensor.matmul(po[:], hcomb[:, trows], rcs[:, ocs],
                                     start=False, stop=True)
                    # half copies overlap the other half's matmuls; one
                    # combined DMA keeps 1KB descriptor lines
                    nc.scalar.activation(ot[:, hs], po[:],
                                         mybir.ActivationFunctionType.Copy,
                                         scale=1.0 / WSC)
                nc.sync.dma_start(out=out_d[trows, ocols], in_=ot[:])

            # phase B: remaining groups. W for n+1 is prefetched right after
            # the first tile of n, so each n-transition never waits on DMA.
            wsub = w0
            for n in range(NT):
                ocols = slice(n * NTILE, (n + 1) * NTILE)
                first = True
                for tt in range(NEARLY + 1 if n == 0 else 0, TT):
                    if n == NT - 1 and tt == TT - 1:
                        last_tile(tt, n, ocols)
                        continue
                    po = psum_out.tile([P, NTILE], f32, tag="po")
                    base_mms(po, tt, wsub)
                    finalize(po, tt, n, ocols)
                    if first and n + 1 < NT:
                        wsub_next = w_subtiles(n + 1)
                        first = False
                wsub = wsub_next

    nc.compile()
    return nc


def _get_nc():
    global _nc_cache
    if _nc_cache is None:
        _nc_cache = _build()
    return _nc_cache


def make_in_maps(x, W, b, A, B, A1, A2, B1, B2):
    """Host-side shard + pack. Returns per-core in_maps for run_bass_kernel_spmd."""
    x = np.ascontiguousarray(np.asarray(x, np.float32)).reshape(NTOK, IN)
    W = np.asarray(W, np.float32)
    b = np.asarray(b, np.float32)
    A = np.asarray(A, np.float32)
    B = np.asarray(B, np.float32)
    A1 = np.asarray(A1, np.float32)
    A2 = np.asarray(A2, np.float32)
    B1 = np.asarray(B1, np.float32)
    B2 = np.asarray(B2, np.float32)

    # W^T [IN, OUT] -> [NT, P, KO, NTILE] so each n-tile DMA is contiguous;
    # scaled by 64 out of e4m3's subnormal range, quantized to fp8
    Wd = np.ascontiguousarray(
        (W.T * WSC).astype(F8).reshape(KO, P, NT, NTILE).transpose(2, 1, 0, 3)
    )
    # cascade fold: B1@A2@A1 [32, IN] in f64, so the chain emits h4 directly;
    # zero-padded to 128 rank rows for full-tile chain matmuls
    AB1 = (B1.astype(np.float64) @ A2.astype(np.float64)
           @ A1.astype(np.float64)).astype(np.float32)
    ATc = np.ascontiguousarray(
        np.concatenate([A.T, AB1.T, np.zeros((IN, 32), np.float32)], axis=1)
        .astype(F16).reshape(KO, P, P).transpose(1, 0, 2)
    )
    Rc = np.zeros((P, OUT), F16)
    Rc[0:64] = (WSC * S1 * B.T).astype(F16)
    Rc[64:96] = (WSC * S2 * B2.T).astype(F16)
    Rc[96] = (WSC * b).astype(F16)

    in_maps = []
    for c in range(N_CORES):
        xs = x[c * TOK:(c + 1) * TOK]                      # [TOK, IN]
        xT = np.ascontiguousarray(
            xs.T.astype(F16).reshape(KO, P, TOK).transpose(1, 0, 2)
        )
        in_maps.append({"xT": xT, "Wd": Wd, "ATc": ATc, "Rc": Rc})
    return in_maps


def kernel(x, W, b, A, B, A1, A2, B1, B2):
    nc = _get_nc()
    in_maps = make_in_maps(x, W, b, A, B, A1, A2, B1, B2)
    res = run_bass_kernel_spmd(nc, in_maps, core_ids=list(range(N_CORES)))
    out = np.concatenate(
        [res.results[c]["out"].astype(np.float32) for c in range(N_CORES)],
        axis=0)
    return out.reshape(4, 2048, OUT)
